# revision 18
# baseline (speedup 1.0000x reference)
"""Trainium2 Bass kernel for 16-head causal MHA (B=2, T=4096, D=1024).

Sharding: 8 cores = 2 batches x 4 head-groups (4 heads each).
Each core computes Q/K/V projections for its 256 cols of Wq/Wk/Wv,
streaming causal attention for its 4 heads, and a partial output
projection against its 256 rows of Wo.  Host sums the 4 partials per
batch and adds the output bias.

Device dataflow (all "transposed", T on the free axis):
  xT    [1024, 4096] bf16  (x[b].T)
  qt/kt 2 SBUF tensors [128, 4096] bf16, packing a head PAIR per tile
        (64 partition rows each).
  vsb   per 128-row key-chunk: 4 heads x [64 V cols | ones col] at
        stride 66 ([V|1] trick: PV matmul row 64 = sum of P = l).
  S^T   per (head-pair, key-chunk): TWO K=64 matmuls (row groups 0-63 /
        64-127 run concurrently on the PE sub-arrays) into one
        [128, 1024] PSUM tile.
  P     one exp activation over both heads' scores ([128, 2, 512-co]
        3D AP), PSUM fp32 -> SBUF bf16. Diagonal triangles masked by a
        DVE multiply with a constant lower-tri mask.
  ctx^T accumulated per head in its own PSUM bank as [65, 512]
        (lhsT = [V|1] stationary 65 cols, P moving N=512): rows 0-63 =
        ctx^T, row 64 = l.  Normalised at the end of each pass by
        broadcasting l (gpsimd partition_broadcast), one
        reciprocal_approx_fast, and two DVE multiplies straight into
        ctxt (pre-transposed for the output projection - no PE
        transposes needed).
  out   o^T [1024, 4096] fp32 = Wo_slice^T @ ctxt, host transposes+sums.

Emission is software-pipelined: scores/exp run LAG=2 chunk groups ahead
of PV so the in-order PE queue never blocks on a just-issued exp, and
projection/output/normalise micro-tasks are dripped between chunk
groups to fill the PE while the ScalarE (exp) streams.
"""

import math

import numpy as np

B, T, D = 2, 4096, 1024
H, HD = 16, 64
NCORES = 8
HPC = 4               # heads per core
DQ = HPC * HD         # 256 per-core projection width
P = 128
TQB = 512             # query block
NQB = T // TQB        # 8
NDC = D // P          # 8 contraction chunks for projections
NTC = T // P          # 32 key/T chunks
VST = 4 * 66          # per-T-chunk V stride: 4 heads * (64 data + 1 one + 1 pad)
LAG = 3               # chunk groups between scores/exp and PV emission
# exp engine schedule per chunk: A=ACT true exp, D=DVE fast-exp (int16
# bitcast bf16, ~3% rms), G=Pool fast-exp (bit-identical to DVE's)
EXP_SCHED = "AD"  # 1:1 ACT:DVE (Pool cannot read PSUM)
MASK_SCHED = "D"      # diagonal-mask multiply engine rotation (pt is in SBUF)

_NC_CACHE = {}


def _build_nc(repeat=1, loop=1):
    import concourse.mybir as mybir
    from concourse import bacc
    from concourse.tile import TileContext

    dt = mybir.dt
    bf = dt.bfloat16
    f32 = dt.float32
    i16 = dt.int16
    AF = mybir.ActivationFunctionType
    ALU = mybir.AluOpType
    # DVE fast-exp: bf16(int16(s*FE_A + FE_B)) ~= exp(s/8) to ~3% rel
    FE_A = 128.0 / math.log(2.0) / 8.0
    FE_B = 127.0 * 128.0 - 5.5 + 0.5

    nc = bacc.Bacc("TRN2", target_bir_lowering=False, debug=False)

    xT = nc.dram_tensor("xT", [D, T], bf, kind="ExternalInput")
    wq = nc.dram_tensor("wq", [D, DQ], bf, kind="ExternalInput")
    wk = nc.dram_tensor("wk", [D, DQ], bf, kind="ExternalInput")
    wv = nc.dram_tensor("wv", [D, DQ], bf, kind="ExternalInput")
    woa = nc.dram_tensor("woa", [P, D], bf, kind="ExternalInput")
    wob = nc.dram_tensor("wob", [P, D], bf, kind="ExternalInput")
    bqk = nc.dram_tensor("bqk", [P, 4], f32, kind="ExternalInput")
    ot = nc.dram_tensor("ot", [D, T], f32, kind="ExternalOutput")

    with TileContext(nc) as tc:
        with (
            tc.tile_pool(name="per", bufs=1) as per,
            tc.tile_pool(name="xp", bufs=2) as xp,
            tc.tile_pool(name="ptp", bufs=4) as ptp,
            tc.tile_pool(name="obp", bufs=3) as obp,
            tc.tile_pool(name="nrm", bufs=2) as nrm,
            tc.tile_pool(name="psS", bufs=2, space="PSUM") as psS,
            tc.tile_pool(name="psC", bufs=1, space="PSUM") as psC,
            tc.tile_pool(name="psM", bufs=2, space="PSUM") as psM,
        ):
            # ---- persistent tensors ----
            qt = [per.tile([P, T], bf, tag=f"qt{i}", name=f"qt{i}") for i in range(2)]
            kt = [per.tile([P, T], bf, tag=f"kt{i}", name=f"kt{i}") for i in range(2)]
            ctxt = [
                per.tile([P, T], bf, tag=f"ctxt{i}", name=f"ctxt{i}") for i in range(2)
            ]
            vsb = per.tile([P, NTC * VST], bf, tag="vsb")
            wq_sb = per.tile([P, NDC * DQ], bf, tag="wq")
            wk_sb = per.tile([P, NDC * DQ], bf, tag="wk")
            wv_sb = per.tile([P, NDC * DQ], bf, tag="wv")
            woa_sb = per.tile([P, D], bf, tag="woa")
            wob_sb = per.tile([P, D], bf, tag="wob")
            bqk_sb = per.tile([P, 4], f32, tag="bqk")
            # lower-tri keep mask, two copies side by side (3D-AP mask of
            # both heads' diagonal tiles in one DVE op)
            tri_sb = per.tile([P, 2 * P], bf, tag="tri")

            # ---- loads ----
            # split the first-needed weights in half so the first Q-proj
            # matmul (contraction chunks 0-3) starts ~halfway into the load,
            # and spread issues across idle engine DMA queues
            _eng = [nc.sync, nc.gpsimd, nc.scalar]
            _ei = [0]

            def _dma(dst, src):
                _eng[_ei[0] % 3].dma_start(dst, src)
                _ei[0] += 1

            def load_weights():
                for w_sb, w_dr in ((wq_sb, wq), (wk_sb, wk), (wv_sb, wv)):
                    for h0, h1 in ((0, 4), (4, NDC)):
                        _dma(
                            w_sb[:].rearrange("p (c n) -> p c n", c=NDC)[:, h0:h1],
                            w_dr[:, :].rearrange("(c p) n -> p c n", p=P)[:, h0:h1],
                        )
                _dma(woa_sb[:], woa[:, :])
                _dma(wob_sb[:], wob[:, :])
                _dma(bqk_sb[:], bqk[:, :])

            # ones columns for the [V|1] trick (col 64 of each 66-stride head
            # block; data cols are fully overwritten by the V copies)
            nc.vector.memset(
                vsb[:].rearrange("p (c x) -> p c x", x=66)[:, :, 64:65], 1.0
            )
            # tri[p, f] = 1 where f >= p (query col sees key row), else 0
            # affine_select keeps in_ where the affine cmp is true, else fill
            nc.gpsimd.memset(tri_sb[:], 1.0)
            nc.gpsimd.affine_select(
                out=tri_sb[:].rearrange("p (h q) -> p h q", h=2),
                in_=tri_sb[:].rearrange("p (h q) -> p h q", h=2),
                compare_op=ALU.is_ge, fill=0.0,
                base=0, pattern=[[0, 2], [1, P]], channel_multiplier=-1,
            )

            gctr = {"n": 0}

            def load_xblk(tb):
                xblk = xp.tile([P, NDC * TQB], bf, tag="xblk", name="xblk")
                nc.sync.dma_start(
                    xblk[:].rearrange("p (c t) -> p c t", c=NDC),
                    xT[:, :].rearrange("(c p) t -> p c t", p=P)[
                        :, :, tb * TQB : (tb + 1) * TQB
                    ],
                )
                return xblk

            xblks = {}

            def proj_tasks(tb):
                """Micro-tasks (~4 matmuls each) projecting block tb.
                Order: Q j0, K j0, V t0..t3, Q j1, K j1.  The xblk DMA is
                issued separately (a full pass earlier) via dma_task."""
                qk_tasks = {0: [], 1: []}
                v_tasks = []
                state = {}

                def get_xblk():
                    return xblks[tb]

                for j in range(2):
                    for w_sb, dst, bcol in ((wq_sb, qt, 0), (wk_sb, kt, 2)):

                        def t_a(w_sb=w_sb, j=j):
                            ps = psM.tile([P, TQB], f32, tag="mix", name="psqk")
                            state["ps"] = ps
                            for d in range(4):
                                nc.tensor.matmul(
                                    ps[:],
                                    w_sb[:, d * DQ + j * P : d * DQ + (j + 1) * P],
                                    get_xblk()[:, d * TQB : (d + 1) * TQB],
                                    start=(d == 0),
                                    stop=False,
                                )

                        def t_b(w_sb=w_sb, dst=dst, bcol=bcol, j=j):
                            ps = state["ps"]
                            for d in range(4, NDC):
                                nc.tensor.matmul(
                                    ps[:],
                                    w_sb[:, d * DQ + j * P : d * DQ + (j + 1) * P],
                                    get_xblk()[:, d * TQB : (d + 1) * TQB],
                                    start=False,
                                    stop=(d == NDC - 1),
                                )
                            nc.vector.tensor_scalar_add(
                                dst[j][:, tb * TQB : (tb + 1) * TQB],
                                ps[:],
                                bqk_sb[:, bcol + j : bcol + j + 1],
                            )

                        qk_tasks[j].append(t_a)
                        qk_tasks[j].append(t_b)

                for t4 in range(4):

                    def v_a(t4=t4):
                        ps = psM.tile([P, TQB], f32, tag="mix", name="psv")
                        state["ps"] = ps
                        for d in range(4):
                            nc.tensor.matmul(
                                ps[:, :DQ],
                                get_xblk()[
                                    :, d * TQB + t4 * P : d * TQB + (t4 + 1) * P
                                ],
                                wv_sb[:, d * DQ : (d + 1) * DQ],
                                start=(d == 0),
                                stop=False,
                            )

                    def v_b(t4=t4):
                        ps = state["ps"]
                        tc_ = tb * 4 + t4
                        for d in range(4, NDC):
                            nc.tensor.matmul(
                                ps[:, :DQ],
                                get_xblk()[
                                    :, d * TQB + t4 * P : d * TQB + (t4 + 1) * P
                                ],
                                wv_sb[:, d * DQ : (d + 1) * DQ],
                                start=False,
                                stop=(d == NDC - 1),
                            )
                        # one strided copy places all 4 heads at stride VST/4
                        nc.vector.tensor_copy(
                            vsb[:, tc_ * VST : tc_ * VST + 4 * 66].rearrange(
                                "p (h d) -> p h d", h=HPC
                            )[:, :, 0:HD],
                            ps[:, :DQ].rearrange("p (h d) -> p h d", d=HD),
                        )

                    v_tasks.append(v_a)
                    v_tasks.append(v_b)

                yield from qk_tasks[0]
                yield from v_tasks
                yield from qk_tasks[1]

            def dma_task(tb):
                def t():
                    xblks[tb] = load_xblk(tb)

                return t

            def outproj_tasks(qb):
                qs = slice(qb * TQB, (qb + 1) * TQB)
                state = {}
                for n in range(NDC):

                    def t(n=n):
                        pso = psM.tile([P, TQB], f32, tag="mix", name="pso")
                        nc.tensor.matmul(
                            pso[:], woa_sb[:, n * P : (n + 1) * P],
                            ctxt[0][:, qs], start=True, stop=False,
                        )
                        nc.tensor.matmul(
                            pso[:], wob_sb[:, n * P : (n + 1) * P],
                            ctxt[1][:, qs], start=False, stop=True,
                        )
                        if n % 2 == 0:
                            osb = obp.tile(
                                [P, 2 * TQB], f32, tag="osb", name="osb"
                            )
                            state["osb"] = osb
                            # alternate PSUM->SBUF copies between DVE and the
                            # Scalar engine to balance load
                            nc.vector.tensor_copy(osb[:, 0:TQB], pso[:])
                        else:
                            osb = state["osb"]
                            nc.scalar.copy(osb[:, TQB:], pso[:])
                            # one DMA stores both 128-row output chunks
                            nc.sync.dma_start(
                                ot[(n - 1) * P : (n + 1) * P, qs].rearrange(
                                    "(c p) t -> p c t", p=P
                                ),
                                osb[:].rearrange("p (c t) -> p c t", c=2),
                            )

                    yield t

            def normalize_tasks(qb, g2, psc):
                """Normalise pass (qb, g2): l rows -> broadcast -> 1/l -> two
                multiplies into ctxt[g2] (even head rows 0-63, odd 64-127)."""
                qs = slice(qb * TQB, (qb + 1) * TQB)

                def t_bcast():
                    lrow = nrm.tile([P, 2 * TQB], f32, tag="lrow", name="lrow")
                    lb = nrm.tile([P, 2 * TQB], f32, tag="lb", name="lb")
                    rb = nrm.tile([P, 2 * TQB], f32, tag="rb", name="rb")
                    # gpsimd cannot read PSUM; stage the l row through SBUF
                    # (cross-partition copy 64 -> 0, validated on HW)
                    nc.vector.tensor_copy(lrow[0:1, :], psc[64:65, :])
                    nc.gpsimd.partition_broadcast(lb[0:64, :], lrow[0:1, :])
                    nc.vector.reciprocal_approx_fast(rb[0:64, :], lb[0:64, :])
                    return rb

                state = {}

                def t0():
                    state["rb"] = t_bcast()

                def t1():
                    rb = state["rb"]
                    nc.vector.tensor_mul(
                        ctxt[g2][0:64, qs], psc[0:64, 0:TQB], rb[0:64, 0:TQB]
                    )

                def t2():
                    rb = state["rb"]
                    tmp = nrm.tile([P, TQB], bf, tag="ntmp", name="ntmp")
                    nc.vector.tensor_mul(tmp[0:64, :], psc[0:64, TQB:], rb[0:64, TQB:])
                    nc.vector.tensor_copy(ctxt[g2][64:128, qs], tmp[0:64, :])

                return [t0, t1, t2]

            def attention_pass(qb, g2, aux, pre=()):
                """Causal attention for heads (2*g2, 2*g2+1) over query block
                qb. `pre` tasks (the previous pass's normalize chain - DVE/
                gpsimd only, they never block the PE queue) are emitted
                eagerly so the first PV's WAR on psc resolves early; aux
                tasks are dripped between chunk groups."""
                for t in pre:
                    t()
                nchunks = 4 * (qb + 1)
                q0 = qb * TQB
                psc = psC.tile([P, 2 * TQB], f32, tag="ctx", name="psc")
                pts = {}

                def scores_exp(ck):
                    tk0 = ck * P
                    co = max(0, tk0 - q0)
                    pss = psS.tile([P, 2 * TQB], f32, tag="scores", name="pss")
                    pt = ptp.tile([P, 2 * TQB], bf, tag="pt", name="pt")
                    for hh in range(2):
                        nc.tensor.matmul(
                            pss[:, hh * TQB + co : (hh + 1) * TQB],
                            kt[g2][hh * 64 : (hh + 1) * 64, tk0 : tk0 + P],
                            qt[g2][hh * 64 : (hh + 1) * 64, q0 + co : q0 + TQB],
                            start=True,
                            stop=True,
                        )
                    src3 = pss[:].rearrange("p (h q) -> p h q", h=2)[:, :, co:TQB]
                    dst3 = pt[:].rearrange("p (h q) -> p h q", h=2)[:, :, co:TQB]
                    gctr["n"] += 1
                    e = EXP_SCHED[gctr["n"] % len(EXP_SCHED)]
                    if e == "D":
                        nc.vector.tensor_scalar(
                            dst3.bitcast(i16), src3, FE_A, FE_B,
                            op0=ALU.mult, op1=ALU.add,
                        )
                    else:
                        nc.scalar.activation(
                            dst3, src3, AF.Exp, scale=1.0 / math.sqrt(HD),
                        )
                    if tk0 >= q0:
                        # diagonal chunk: zero the upper-left triangle of both
                        # heads in one multiply with the lower-tri mask
                        tgt = pt[:].rearrange("p (h q) -> p h q", h=2)[
                            :, :, co : co + P
                        ]
                        gctr["m"] = gctr.get("m", 0) + 1
                        meng = (
                            nc.vector
                            if MASK_SCHED[gctr["m"] % len(MASK_SCHED)] == "D"
                            else nc.gpsimd
                        )
                        meng.tensor_mul(
                            tgt, tgt, tri_sb[:].rearrange("p (h q) -> p h q", h=2)
                        )
                    pts[ck] = (pt, co)

                def pv(ck):
                    pt, co = pts.pop(ck)
                    tc_ = ck
                    for hh in range(2):
                        h = 2 * g2 + hh
                        nc.tensor.matmul(
                            psc[0:65, hh * TQB + co : (hh + 1) * TQB],
                            vsb[:, tc_ * VST + h * 66 : tc_ * VST + h * 66 + 65],
                            pt[:, hh * TQB + co : (hh + 1) * TQB],
                            start=(ck == 0),
                            stop=(ck == nchunks - 1),
                        )

                emitted = 0
                done = 0
                total = nchunks + 1
                for ck in range(nchunks):
                    scores_exp(ck)
                    if ck >= LAG:
                        pv(ck - LAG)
                    done += 1
                    want = (done * len(aux)) // total
                    while emitted < want:
                        aux[emitted]()
                        emitted += 1
                for ck in range(max(0, nchunks - LAG), nchunks):
                    pv(ck)
                while emitted < len(aux):
                    aux[emitted]()
                    emitted += 1
                return psc

            # ---- software-pipelined schedule ----
            import contextlib

            _hint = (
                mybir.EngineType.PE,
                mybir.EngineType.Activation,
                mybir.EngineType.DVE,
                mybir.EngineType.Pool,
                mybir.EngineType.SP,
            )
            loop_cm = (
                tc.For_i(0, loop, 1, hint_engines=_hint)
                if loop > 1
                else contextlib.nullcontext()
            )
            # first x-block DMA heads the SP queue ahead of the weight loads
            # (emission order = scheduler priority)
            dma_task(0)()
            load_weights()
            dma_task(1)()

            with loop_cm:
              for _rep in range(repeat):
                if _rep:
                    dma_task(0)()
                    dma_task(1)()
                # bootstrap: only Q j0 + K j0 gate the first scores; the V
                # and j1 projections overlap with the first attention pass
                pj0 = list(proj_tasks(0))
                for t in pj0[0:4]:
                    t()
                boot = pj0[4:]
                prev_norm = None  # normalize tasks of the previous pass
                for qb in range(NQB):
                    for g2 in range(2):
                        aux = []
                        pre = prev_norm or ()
                        if g2 == 0:
                            if qb == 0:
                                aux += boot
                            if qb + 1 < NQB:
                                pj = list(proj_tasks(qb + 1))
                                # Q j0 + K j0 first half
                                aux += pj[0:4]
                                rest = pj[4:]
                            else:
                                rest = []
                            if qb > 0:
                                aux += list(outproj_tasks(qb - 1))
                            state_rest = rest
                        else:
                            aux += state_rest
                            if qb + 2 < NQB:
                                aux.append(dma_task(qb + 2))
                        psc = attention_pass(qb, g2, aux, pre)
                        prev_norm = normalize_tasks(qb, g2, psc)
                for t in prev_norm:
                    t()
                for t in outproj_tasks(NQB - 1):
                    t()

    nc.compile()
    return nc


def _get_nc():
    if "nc" not in _NC_CACHE:
        _NC_CACHE["nc"] = _build_nc()
    return _NC_CACHE["nc"]


def _in_maps(x, Wq, bq, Wk, bk, Wv, bv, Wo, bo):
    import ml_dtypes

    bf = ml_dtypes.bfloat16
    maps = []
    for c in range(NCORES):
        b, hg = divmod(c, 4)
        cs = slice(hg * DQ, (hg + 1) * DQ)
        bqk_pack = np.stack(
            [
                bq[cs][0:128], bq[cs][128:256],
                bk[cs][0:128], bk[cs][128:256],
            ],
            axis=1,
        ).astype(np.float32)
        maps.append(
            {
                "xT": np.ascontiguousarray(x[b].T).astype(bf),
                "wq": Wq[:, cs].astype(bf),
                "wk": Wk[:, cs].astype(bf),
                "wv": Wv[:, cs].astype(bf),
                "woa": Wo[cs, :][0:128].astype(bf),
                "wob": Wo[cs, :][128:256].astype(bf),
                "bqk": np.ascontiguousarray(bqk_pack),
            }
        )
    return maps


def kernel(x, Wq, bq, Wk, bk, Wv, bv, Wo, bo):
    from concourse.bass_utils import run_bass_kernel_spmd

    nc = _get_nc()
    maps = _in_maps(x, Wq, bq, Wk, bk, Wv, bv, Wo, bo)
    res = run_bass_kernel_spmd(nc, maps, list(range(NCORES)))
    # bv never reaches the device: softmax rows sum to 1, so
    # ctx = P@(x@Wv)/l + bv exactly, and bv@Wo folds into the output bias.
    bo_eff = (bo.astype(np.float64) + bv.astype(np.float64) @ Wo.astype(np.float64)).astype(np.float32)
    out = np.zeros((B, T, D), np.float32)
    for b in range(B):
        acc = res.results[b * 4]["ot"].astype(np.float32)
        for g in range(1, 4):
            acc = acc + res.results[b * 4 + g]["ot"]
        out[b] = acc.T + bo_eff
    return out

